# revision 27
# baseline (speedup 1.0000x reference)
"""Multi-head attention + output dense + LayerNorm + residual, on 8 NeuronCores.

Sharding: core c -> (batch b = c//2, query-half hf = c%2). Each core runs the
full 16-head attention for its 1024 queries against its batch's full 2048
keys. K/V projections are computed for each core's OWN 1024 tokens only and
exchanged between the two cores of a batch with an AllGather over replica
groups [[0,1],[2,3],[4,5],[6,7]] -- rank order == global token order, so the
gathered K/V are globally ordered and the device program stays SPMD-uniform
(key order is softmax-invariant; only queries need the host-side reorder).

Bass kernel layout choices:
  - All matmul inputs are bf16 (fp32 PSUM accumulation); weights are cast to
    bf16 on the host, activations are cast on-chip.
  - Q'^T / K'^T tiles are [128, q] with partitions = (2 heads x 64 dims), so
    the scores matmul is a clean K=64 contraction. The two heads of a pair
    run CONCURRENTLY in the PE array via tile_position row tiling
    ((0,0) and (64,0)), doubling scores throughput (HW-verified 2x).
  - The attention mask enters MULTIPLICATIVELY through V: probs = e^m * e^s,
    so V rows are scaled by e^{m_k} and the denominator ones-row of v_all
    holds e^{m_k}. Exact, and it frees the 65th contraction row.
  - exp of DVE_KC key chunks runs on the vector engine as a Schraudolph
    int16 exp (i16 = A*s + B, bitcast to bf16 == 2^(i/128-127)), the rest on
    ScalarE -- the softmax exp is split across two engines. The Schraudolph
    bias constant is centered so both paths agree in scale; the residual
    sawtooth (~1.8% rms) is well inside the 2e-2 gate (measured 6.7e-3).
  - ctx lags scores by 3 chunks; each block's recip-broadcast matmul and
    final scale are deferred into the next block's stream (dense PE queue).
  - All bias-fold matmuls are gone: bv folds into bd_eff = bd + bv @ Wd on
    the host (exact: probs sum to 1), added during the output-projection
    PSUM->SBUF copy; bq/bk ride the projection copies on ScalarE.
"""

import numpy as np

B, S, H, NH = 4, 2048, 1024, 16
HD = H // NH  # 64
SQ = S // 2  # queries per core
NCORES = 8
NPAIR = NH // 2  # head pairs
NCI = H // 128  # 8 contraction chunks
NKC = S // 128  # 16 key chunks
NKCH = NKC // 2  # 8 key chunks per core before the exchange
EPS = 1e-12

# Schraudolph int16 exp on DVE for these key chunks (the rest use ScalarE's
# exact exp): i16 = round(A * s_raw + B); bitcast int16 -> bf16 gives
# 2^(i/128 - 127) ~ e^(s_raw/8). B centers the 1+f vs 2^f interpolation bias
# so the two engines' probs agree in scale (the rest cancels in the softmax
# denominator).
DVE_KC = frozenset({2, 4, 7, 9, 12, 14})
SCHR_A = 16.0 * float(np.log2(np.e))  # 23.0831...
SCHR_B = 16256.0 - 7.4 + 0.5  # +0.5: truncation -> round-to-nearest

_cache = {}
_DEBUG_TAPS = False


def _build():
    import concourse.bass as bass
    import concourse.bacc as bacc
    import concourse.mybir as mybir
    import concourse.tile as tile

    fp32 = mybir.dt.float32
    bf16 = mybir.dt.bfloat16
    int16 = mybir.dt.int16
    AF = mybir.ActivationFunctionType
    OP = mybir.AluOpType

    nc = bacc.Bacc("TRN2", target_bir_lowering=False, debug=False, num_devices=8)

    xkv = nc.dram_tensor("xkv", [SQ, H], fp32, kind="ExternalInput").ap()
    xtb_d = nc.dram_tensor("xtb", [NKCH, 128, NCI, 128], bf16, kind="ExternalInput").ap()
    expmo_d = nc.dram_tensor("expmo", [SQ], fp32, kind="ExternalInput").ap()
    wq_d = nc.dram_tensor("wq", [H, H], bf16, kind="ExternalInput").ap()
    wk_d = nc.dram_tensor("wk", [H, H], bf16, kind="ExternalInput").ap()
    wv_d = nc.dram_tensor("wv", [H, H], bf16, kind="ExternalInput").ap()
    wd_d = nc.dram_tensor("wd", [H, H], bf16, kind="ExternalInput").ap()
    bq_d = nc.dram_tensor("bq", [H], fp32, kind="ExternalInput").ap()
    bk_d = nc.dram_tensor("bk", [H], fp32, kind="ExternalInput").ap()
    bd_d = nc.dram_tensor("bd", [H], fp32, kind="ExternalInput").ap()
    gamma_d = nc.dram_tensor("gamma", [H], fp32, kind="ExternalInput").ap()
    out_d = nc.dram_tensor("out", [SQ, H], fp32, kind="ExternalOutput").ap()

    KCOLS = NCI * SQ  # bf16 columns of the K half in the exchange buffer
    VCOLS = NKCH * NH * 65  # V half columns (incl. the e^mask denominator col)
    CC_GROUPS = [[0, 1], [2, 3], [4, 5], [6, 7]]

    with tile.TileContext(nc) as tc:
        with (
            tc.tile_pool(name="consts", bufs=1) as consts,
            tc.tile_pool(name="ctxT", bufs=1) as ctxt_pool,
            tc.tile_pool(name="ccdram", bufs=1, space="DRAM") as ccdram,
        ):
            # --- constants ---
            bqT = consts.tile([128, NPAIR], fp32)
            nc.gpsimd.dma_start(out=bqT, in_=bq_d.rearrange("(c p) -> p c", p=128))
            bkT = consts.tile([128, NCI], fp32)
            nc.gpsimd.dma_start(out=bkT, in_=bk_d.rearrange("(c p) -> p c", p=128))

            def bcast128(ap):
                return bass.AP(tensor=ap.tensor, offset=ap.offset, ap=[[0, 128]] + list(ap.ap))

            gamma_b = consts.tile([128, H], fp32)
            nc.gpsimd.dma_start(out=gamma_b, in_=bcast128(gamma_d))
            bd128 = consts.tile([128, H], fp32)
            nc.gpsimd.dma_start(out=bd128, in_=bcast128(bd_d))
            expmo_sb = consts.tile([128, NKCH], fp32)
            nc.gpsimd.dma_start(out=expmo_sb, in_=expmo_d.rearrange("(c p) -> p c", p=128))
            eps_sb = consts.tile([128, 1], fp32)
            nc.vector.memset(eps_sb, EPS)
            # broadcast selector: row 0 -> out partitions 0:64, row 64 -> 64:128.
            # 65-deep so the broadcast matmul stays in fast full-tile mode.
            sel65 = consts.tile([65, 128], bf16)
            nc.vector.memset(sel65, 0.0)
            nc.vector.memset(sel65[0:1, 0:64], 1.0)
            nc.vector.memset(sel65[64:65, 64:128], 1.0)
            # two recip carriers, alternated per attention block so the
            # deferred broadcast matmul of block i never WAR-blocks block i+1
            recip65s = []
            for i in range(2):
                r = consts.tile([65, 512], bf16, name=f"recip65_{i}")
                nc.vector.memset(r, 0.0)
                recip65s.append(r)

            # ctxT[hl*64+d, hp, q] = ctx[q, (hp*2+hl)*64+d] / sumexp
            ctxt = ctxt_pool.tile([128, NPAIR, SQ], bf16)
            wd_sb = ctxt_pool.tile([128, NCI, H], bf16, name="wd_sb")

            ctx_mid = tc.tile_pool(name="midA", bufs=1)
            midA = ctx_mid.__enter__()
            xt = midA.tile([128, NCI, SQ], bf16, name="xt")
            wq_full = midA.tile([128, NCI, H], bf16, name="wq_full")
            ktp_full = midA.tile([128, NCI, S], bf16, name="ktp_full")
            qtp_all = midA.tile([128, NPAIR, SQ], bf16, name="qtp_all")
            v_all = midA.tile([128, NKC, NH, 65], bf16, name="v_all")
            # denominator column for OWN tokens: e^{mask_t}, all heads (the
            # gathered halves carry their owners' e^m columns through the
            # exchange, so no post-gather fill is needed)
            nc.gpsimd.tensor_copy(
                out=v_all[:, 0:NKCH, :, 64:65].rearrange("p a b c -> p a (b c)"),
                in_=bass.AP(
                    tensor=expmo_sb.tensor,
                    offset=expmo_sb.offset,
                    ap=list(expmo_sb.ap) + [[0, NH]],
                ),
            )

            # exchange buffers (K half then V half, flat bf16)
            cc_in = ccdram.tile([128, KCOLS + VCOLS], bf16)
            cc_out = ccdram.tile([2, 128, KCOLS + VCOLS], bf16)

            # --- phase 1: K/V projections for OWN tokens + exchange ---
            with (
                tc.tile_pool(name="wvf", bufs=1) as wvf_pool,
                tc.tile_pool(name="vp", bufs=2, space="PSUM") as vp,
            ):
                wv_full = wvf_pool.tile([128, NCI, H], bf16, name="wv_full")
                wk_full = wvf_pool.tile([128, NCI, H], bf16, name="wk_full")
                wv_re = wv_d.rearrange("(c p) n -> p c n", p=128)
                nc.sync.dma_start(out=wv_full[:, :, 0:512], in_=wv_re[:, :, 0:512])
                for tch in range(NKCH):
                    eng = (nc.sync, nc.scalar)[tch % 2]
                    eng.dma_start(
                        out=xt[:, :, tch * 128 : (tch + 1) * 128], in_=xtb_d[tch]
                    )
                nc.scalar.dma_start(out=wv_full[:, :, 512:H], in_=wv_re[:, :, 512:H])
                nc.sync.dma_start(
                    out=wq_full, in_=wq_d.rearrange("(c p) n -> p c n", p=128)
                )
                nc.scalar.dma_start(
                    out=wk_full, in_=wk_d.rearrange("(c p) n -> p c n", p=128)
                )
                nc.sync.dma_start(
                    out=wd_sb, in_=wd_d.rearrange("(c p) n -> p c n", p=128)
                )
                # V for own tokens: v_all[t, tb, h, 0:64] *= e^{m_t}
                for nh in range(2):
                    for tb in range(NKCH):
                        pv = vp.tile([128, 512], fp32)
                        for ci in range(NCI):
                            nc.tensor.matmul(
                                pv,
                                xt[:, ci, tb * 128 : (tb + 1) * 128],
                                wv_full[:, ci, nh * 512 : (nh + 1) * 512],
                                start=(ci == 0),
                                stop=(ci == NCI - 1),
                            )
                        nc.vector.tensor_scalar_mul(
                            out=v_all[:, tb, nh * 8 : (nh + 1) * 8, 0:64],
                            in0=pv.rearrange("p (a b) -> p a b", a=8),
                            scalar1=expmo_sb[:, tb : tb + 1],
                        )
                # K'^T for own tokens: ktp_full[:, ci, 0:1024]
                for ci in range(NCI):
                    for tb in range(2):
                        pk = vp.tile([128, 512], fp32)
                        for c2 in range(NCI):
                            nc.tensor.matmul(
                                pk,
                                wk_full[:, c2, ci * 128 : (ci + 1) * 128],
                                xt[:, c2, tb * 512 : (tb + 1) * 512],
                                start=(c2 == 0),
                                stop=(c2 == NCI - 1),
                            )
                        nc.scalar.activation(
                            out=ktp_full[:, ci, tb * 512 : (tb + 1) * 512],
                            in_=pk,
                            func=AF.Identity,
                            bias=bkT[:, ci : ci + 1],
                        )
                # ship own halves, gather both (rank order == global order)
                nc.gpsimd.dma_start(
                    out=cc_in[:, 0:KCOLS].rearrange("p (c t) -> p c t", c=NCI),
                    in_=ktp_full[:, :, 0:SQ],
                )
                nc.gpsimd.dma_start(
                    out=cc_in[:, KCOLS:].rearrange("p (k h d) -> p k h d", k=NKCH, h=NH),
                    in_=v_all[:, 0:NKCH, :, :],
                )
                nc.gpsimd.collective_compute(
                    "AllGather",
                    mybir.AluOpType.bypass,
                    replica_groups=CC_GROUPS,
                    ins=[cc_in.opt()],
                    outs=[cc_out.opt()],
                )
                for r in range(2):
                    nc.sync.dma_start(
                        out=ktp_full[:, :, r * SQ : (r + 1) * SQ],
                        in_=cc_out[r, :, 0:KCOLS].rearrange("p (c t) -> p c t", c=NCI),
                    )
                    nc.scalar.dma_start(
                        out=v_all[:, r * NKCH : (r + 1) * NKCH, :, :],
                        in_=cc_out[r, :, KCOLS:].rearrange(
                            "p (k h d) -> p k h d", k=NKCH, h=NH
                        ),
                    )
                # Q projections for all head pairs (overlaps the gather)
                for hp in range(NPAIR):
                    for qb in range(SQ // 512):
                        pq = vp.tile([128, 512], fp32)
                        for ci in range(NCI):
                            nc.tensor.matmul(
                                pq,
                                wq_full[:, ci, hp * 128 : (hp + 1) * 128],
                                xt[:, ci, qb * 512 : (qb + 1) * 512],
                                start=(ci == 0),
                                stop=(ci == NCI - 1),
                            )
                        nc.scalar.activation(
                            out=qtp_all[:, hp, qb * 512 : (qb + 1) * 512],
                            in_=pq,
                            func=AF.Identity,
                            bias=bqT[:, hp : hp + 1],
                        )

            # --- phase 2: attention per head pair ---
            with (
                tc.tile_pool(name="exps", bufs=4) as exps_pool,
                tc.tile_pool(name="exps16", bufs=4) as exps16_pool,
                tc.tile_pool(name="recips", bufs=2) as recips_pool,
                tc.tile_pool(name="pp", bufs=2, space="PSUM") as pp,
                tc.tile_pool(name="sp", bufs=2, space="PSUM") as sp,
                tc.tile_pool(name="cp", bufs=1, space="PSUM") as cp,
            ):
                pending_norm = []
                for hp in range(NPAIR):
                    # the two heads run concurrently in the PE array (row
                    # groups 0-1 vs 2-3 via tile_position auto-derivation);
                    # ctx lags scores by 3 chunks; normalize part 2 of each
                    # block is deferred into the next block's stream.
                    for qb in range(SQ // 512):
                        qsl = slice(qb * 512, (qb + 1) * 512)
                        pcA = cp.tile([65, 512], fp32, tag="pcA")
                        pcB = cp.tile([65, 512], fp32, tag="pcB")
                        es_q = {}

                        def ctx_pair(kc):
                            for hl in range(2):
                                nc.tensor.matmul(
                                    pcA if hl == 0 else pcB,
                                    v_all[:, kc, hp * 2 + hl, :],
                                    es_q[kc][:, hl, :],
                                    start=(kc == 0),
                                    stop=(kc == NKC - 1),
                                )
                            del es_q[kc]

                        for kc in range(NKC):
                            kcsl = slice(kc * 128, (kc + 1) * 128)
                            ps = sp.tile([128, 2, 512], fp32)
                            nc.tensor.matmul(
                                ps[:, 0, :],
                                ktp_full[0:64, hp, kcsl],
                                qtp_all[0:64, hp, qsl],
                                start=True,
                                stop=True,
                            )
                            nc.tensor.matmul(
                                ps[:, 1, :],
                                ktp_full[64:128, hp, kcsl],
                                qtp_all[64:128, hp, qsl],
                                start=True,
                                stop=True,
                            )
                            if kc == 2 and pending_norm:
                                pending_norm.pop()()
                            if kc in DVE_KC:
                                es16 = exps16_pool.tile([128, 2, 512], int16)
                                nc.vector.tensor_scalar(
                                    out=es16.rearrange("p a b -> p (a b)"),
                                    in0=ps.rearrange("p a b -> p (a b)"),
                                    scalar1=SCHR_A,
                                    scalar2=SCHR_B,
                                    op0=OP.mult,
                                    op1=OP.add,
                                )
                                es = es16.bitcast(bf16)
                            else:
                                es = exps_pool.tile([128, 2, 512], bf16)
                                nc.scalar.activation(
                                    out=es.rearrange("p a b -> p (a b)"),
                                    in_=ps.rearrange("p a b -> p (a b)"),
                                    func=AF.Exp,
                                    scale=0.125,
                                )
                            es_q[kc] = es
                            if kc >= 3:
                                ctx_pair(kc - 3)
                        ctx_pair(NKC - 3)
                        ctx_pair(NKC - 2)
                        ctx_pair(NKC - 1)
                        # normalize, part 1 (immediate -- drains pcA/pcB):
                        # denominator rows to SBUF, reciprocal, fill the recip
                        # carrier; ctx rows copied to ctxt on the scalar engine.
                        # (custom-DVE reciprocal needs SBUF base-partition-0
                        # operands on HW -- copy the PSUM denominator row first.)
                        r65 = recip65s[(hp * 2 + qb) % 2]
                        sumsA = recips_pool.tile([1, 512], fp32, tag="sumsA")
                        sumsB = recips_pool.tile([1, 512], fp32, tag="sumsB")
                        nc.vector.tensor_copy(out=sumsA, in_=pcA[64:65, :])
                        nc.vector.tensor_copy(out=sumsB, in_=pcB[64:65, :])
                        recipA = recips_pool.tile([1, 512], fp32, tag="recipA")
                        recipB = recips_pool.tile([1, 512], fp32, tag="recipB")
                        nc.vector.reciprocal_approx_fast(out=recipA, in_=sumsA)
                        nc.vector.reciprocal_approx_fast(out=recipB, in_=sumsB)
                        nc.vector.tensor_copy(out=r65[0:1, :], in_=recipA)
                        nc.vector.tensor_copy(out=r65[64:65, :], in_=recipB)
                        nc.scalar.copy(out=ctxt[0:64, hp, qsl], in_=pcA[0:64, :])
                        nc.scalar.copy(out=ctxt[64:128, hp, qsl], in_=pcB[0:64, :])

                        # normalize, part 2 (deferred): broadcast 1/sum to 128
                        # partitions via one 65-deep matmul, then scale ctxt
                        def norm2(hp=hp, qsl=qsl, r65=r65):
                            pb = pp.tile([128, 512], fp32, tag="proj")
                            nc.tensor.matmul(pb, sel65, r65, start=True, stop=True)
                            nc.vector.tensor_mul(
                                ctxt[:, hp, qsl], ctxt[:, hp, qsl], pb
                            )

                        pending_norm.append(norm2)
                if pending_norm:
                    pending_norm.pop()()

            ctx_mid.__exit__(None, None, None)

            # --- phase 3: output projection + LayerNorm + residual ---
            with (
                tc.tile_pool(name="hid", bufs=3) as hid_pool,
                tc.tile_pool(name="lnbuf", bufs=3) as lnbuf,
                tc.tile_pool(name="op", bufs=3, space="PSUM") as op_pool,
            ):
                for qt in range(SQ // 128):
                    qsl = slice(qt * 128, (qt + 1) * 128)
                    hid = hid_pool.tile([128, H], fp32)
                    for nb in range(2):
                        po = op_pool.tile([128, 512], fp32)
                        for ci in range(NCI):
                            nc.tensor.matmul(
                                po,
                                ctxt[:, ci, qsl],
                                wd_sb[:, ci, nb * 512 : (nb + 1) * 512],
                                start=(ci == 0),
                                stop=(ci == NCI - 1),
                            )
                        nc.vector.tensor_tensor(
                            out=hid[:, nb * 512 : (nb + 1) * 512],
                            in0=po,
                            in1=bd128[:, nb * 512 : (nb + 1) * 512],
                            op=OP.add,
                        )
                    # LayerNorm stats
                    stats = lnbuf.tile([128, 2, 6], fp32, tag="stats")
                    for sg in range(2):
                        nc.vector.bn_stats(
                            out=stats[:, sg, :], in_=hid[:, sg * 512 : (sg + 1) * 512]
                        )
                    mv = lnbuf.tile([128, 2], fp32, tag="mv")
                    nc.vector.bn_aggr(out=mv, in_=stats)
                    rstd = lnbuf.tile([128, 1], fp32, tag="rstd")
                    nc.scalar.activation(
                        out=rstd, in_=mv[:, 1:2], func=AF.Sqrt, bias=eps_sb
                    )
                    nc.vector.reciprocal(rstd, rstd)
                    # residual (beta pre-folded into xkv on the host:
                    # norm + (x + beta) == norm + beta + x exactly)
                    x_res = lnbuf.tile([128, H], fp32, tag="xres")
                    nc.sync.dma_start(out=x_res, in_=xkv[qsl, :])
                    # (hid - mu) * rstd * gamma + (x + beta)
                    norm = lnbuf.tile([128, H], fp32, tag="norm")
                    nc.vector.tensor_scalar(
                        out=norm,
                        in0=hid,
                        scalar1=mv[:, 0:1],
                        scalar2=rstd,
                        op0=OP.subtract,
                        op1=OP.mult,
                    )
                    nc.vector.tensor_mul(norm, norm, gamma_b)
                    final = lnbuf.tile([128, H], fp32, tag="final")
                    nc.gpsimd.tensor_tensor(out=final, in0=norm, in1=x_res, op=OP.add)
                    nc.sync.dma_start(out=out_d[qsl, :], in_=final)

    nc.compile()
    return nc


def get_nc():
    if "nc" not in _cache:
        _cache["nc"] = _build()
    return _cache["nc"]


def make_in_maps(inputs):
    q = np.ascontiguousarray(np.asarray(inputs["query"], dtype=np.float32))
    am = np.asarray(inputs["attention_mask"], dtype=np.float32).reshape(B, S)
    import ml_dtypes

    bfl = ml_dtypes.bfloat16
    # bv folds exactly into the output bias: sum of probs is 1, so
    # (ctx + bv) @ Wd + bd == ctx @ Wd + (bv @ Wd + bd). Computed in fp64.
    bd_eff = (
        np.asarray(inputs["bd"], np.float64)
        + np.asarray(inputs["bv"], np.float64) @ np.asarray(inputs["Wd"], np.float64)
    ).astype(np.float32)
    shared = {
        "wq": np.ascontiguousarray(np.asarray(inputs["Wq"], np.float32).astype(bfl)),
        "wk": np.ascontiguousarray(np.asarray(inputs["Wk"], np.float32).astype(bfl)),
        "wv": np.ascontiguousarray(np.asarray(inputs["Wv"], np.float32).astype(bfl)),
        "wd": np.ascontiguousarray(np.asarray(inputs["Wd"], np.float32).astype(bfl)),
        "bq": np.asarray(inputs["bq"], np.float32),
        "bk": np.asarray(inputs["bk"], np.float32),
        "bd": bd_eff,
        "gamma": np.asarray(inputs["ln_gamma"], np.float32),
    }
    beta_h = np.asarray(inputs["ln_beta"], np.float32)[None, :]
    in_maps = []
    for c in range(NCORES):
        b, hf = c // 2, c % 2
        # own tokens = own queries (global half hf); keys stay global-ordered
        xq = q[b, hf * SQ : (hf + 1) * SQ]
        expg = np.exp(am[b].astype(np.float64)).astype(np.float32)
        m = dict(shared)
        m["xkv"] = np.ascontiguousarray(xq + beta_h)
        xtc = xq.reshape(SQ // 128, 128, H // 128, 128).transpose(0, 3, 2, 1)
        m["xtb"] = np.ascontiguousarray(xtc.astype(bfl))
        m["expmo"] = np.ascontiguousarray(expg[hf * SQ : (hf + 1) * SQ])
        in_maps.append(m)
    return in_maps


def assemble(results):
    out = np.empty((B, S, H), dtype=np.float32)
    for c in range(NCORES):
        b, hf = c // 2, c % 2
        out[b, hf * SQ : (hf + 1) * SQ, :] = results[c]["out"]
    return out


def kernel(**inputs):
    from concourse.bass_utils import run_bass_kernel_spmd

    nc = get_nc()
    in_maps = make_in_maps(inputs)
    res = run_bass_kernel_spmd(nc, in_maps, core_ids=list(range(NCORES)))
    return assemble(res.results)


if __name__ == "__main__":
    rng = np.random.default_rng(0)
    inputs = {
        "query": rng.standard_normal((B, S, H), dtype=np.float32),
        "attention_mask": np.zeros((B, 1, 1, S), np.float32),
        "Wq": rng.standard_normal((H, H), dtype=np.float32) * 0.02,
        "bq": np.zeros(H, np.float32),
        "Wk": rng.standard_normal((H, H), dtype=np.float32) * 0.02,
        "bk": np.zeros(H, np.float32),
        "Wv": rng.standard_normal((H, H), dtype=np.float32) * 0.02,
        "bv": np.zeros(H, np.float32),
        "Wd": rng.standard_normal((H, H), dtype=np.float32) * 0.02,
        "bd": np.zeros(H, np.float32),
        "ln_gamma": np.ones(H, np.float32),
        "ln_beta": np.zeros(H, np.float32),
    }
    out = kernel(**inputs)
    print(out.shape, out.dtype)


# revision 31
# speedup vs baseline: 1.4469x; 1.4469x over previous
"""Multi-head attention + output dense + LayerNorm + residual, on 8 NeuronCores.

Sharding: core c -> (batch b = c//2, query-half hf = c%2). Each core runs the
full 16-head attention for its 1024 queries against its batch's full 2048
keys (K/V projections are recomputed per query-half; an AllGather exchange
was tried and measured 128us -- slower than recomputing). The host reorders
tokens so each core's queries are rows 0:1024 of its slab; key order is
softmax-invariant, so the device program is completely SPMD-uniform.
K(ci>0)/Q(hp>0) projection groups are interleaved INTO the attention kc-loop
as PE filler so exp-latency never starves the PE queue.

Bass kernel layout choices:
  - All matmul inputs are bf16 (fp32 PSUM accumulation); weights are cast to
    bf16 on the host, activations are cast on-chip.
  - Q'^T / K'^T tiles are [128, q] with partitions = (2 heads x 64 dims), so
    the scores matmul is a clean K=64 contraction. The two heads of a pair
    run CONCURRENTLY in the PE array via tile_position row tiling
    ((0,0) and (64,0)), doubling scores throughput (HW-verified 2x).
  - The attention mask enters MULTIPLICATIVELY through V: probs = e^m * e^s,
    so V rows are scaled by e^{m_k} and the denominator ones-row of v_all
    holds e^{m_k}. Exact, and it frees the 65th contraction row.
  - exp of DVE_KC key chunks runs on the vector engine as a Schraudolph
    int16 exp (i16 = A*s + B, bitcast to bf16 == 2^(i/128-127)), the rest on
    ScalarE -- the softmax exp is split across two engines. The Schraudolph
    bias constant is centered so both paths agree in scale; the residual
    sawtooth (~1.8% rms) is well inside the 2e-2 gate (measured 6.7e-3).
  - ctx lags scores by 3 chunks; each block's recip-broadcast matmul and
    final scale are deferred into the next block's stream (dense PE queue).
  - All bias-fold matmuls are gone: bv folds into bd_eff = bd + bv @ Wd on
    the host (exact: probs sum to 1), added during the output-projection
    PSUM->SBUF copy; bq/bk ride the projection copies on ScalarE.
"""

import numpy as np

B, S, H, NH = 4, 2048, 1024, 16
HD = H // NH  # 64
SQ = S // 2  # queries per core
NCORES = 8
NPAIR = NH // 2  # head pairs
NCI = H // 128  # 8 contraction chunks
NKC = S // 128  # 16 key chunks
NKCH = NKC // 2  # 8 key chunks per core before the exchange
EPS = 1e-12

# Schraudolph int16 exp on DVE for these key chunks (the rest use ScalarE's
# exact exp): i16 = round(A * s_raw + B); bitcast int16 -> bf16 gives
# 2^(i/128 - 127) ~ e^(s_raw/8). B centers the 1+f vs 2^f interpolation bias
# so the two engines' probs agree in scale (the rest cancels in the softmax
# denominator).
DVE_KC = frozenset({2, 4, 7, 9, 12, 14})
SCHR_A = 16.0 * float(np.log2(np.e))  # 23.0831...
SCHR_B = 16256.0 - 7.4 + 0.5  # +0.5: truncation -> round-to-nearest

_cache = {}
_DEBUG_TAPS = False


def _build():
    import concourse.bass as bass
    import concourse.bacc as bacc
    import concourse.mybir as mybir
    import concourse.tile as tile

    fp32 = mybir.dt.float32
    bf16 = mybir.dt.bfloat16
    int16 = mybir.dt.int16
    AF = mybir.ActivationFunctionType
    OP = mybir.AluOpType

    nc = bacc.Bacc("TRN2", target_bir_lowering=False, debug=False, num_devices=8)

    xkv = nc.dram_tensor("xkv", [SQ, H], fp32, kind="ExternalInput").ap()
    xtb_d = nc.dram_tensor("xtb", [NKC, 128, NCI, 128], bf16, kind="ExternalInput").ap()
    expm_d = nc.dram_tensor("expm", [S], fp32, kind="ExternalInput").ap()
    wq_d = nc.dram_tensor("wq", [H, H], bf16, kind="ExternalInput").ap()
    wk_d = nc.dram_tensor("wk", [H, H], bf16, kind="ExternalInput").ap()
    wv_d = nc.dram_tensor("wv", [H, H], bf16, kind="ExternalInput").ap()
    wd_d = nc.dram_tensor("wd", [H, H], bf16, kind="ExternalInput").ap()
    bq_d = nc.dram_tensor("bq", [H], fp32, kind="ExternalInput").ap()
    bk_d = nc.dram_tensor("bk", [H], fp32, kind="ExternalInput").ap()
    bd_d = nc.dram_tensor("bd", [H], fp32, kind="ExternalInput").ap()
    gamma_d = nc.dram_tensor("gamma", [H], fp32, kind="ExternalInput").ap()
    out_d = nc.dram_tensor("out", [SQ, H], fp32, kind="ExternalOutput").ap()

    with tile.TileContext(nc) as tc:
        with (
            tc.tile_pool(name="consts", bufs=1) as consts,
            tc.tile_pool(name="ctxT", bufs=1) as ctxt_pool,
        ):
            # --- constants ---
            bqT = consts.tile([128, NPAIR], fp32)
            nc.gpsimd.dma_start(out=bqT, in_=bq_d.rearrange("(c p) -> p c", p=128))
            bkT = consts.tile([128, NCI], fp32)
            nc.gpsimd.dma_start(out=bkT, in_=bk_d.rearrange("(c p) -> p c", p=128))

            def bcast128(ap):
                return bass.AP(tensor=ap.tensor, offset=ap.offset, ap=[[0, 128]] + list(ap.ap))

            gamma_b = consts.tile([128, H], fp32)
            nc.gpsimd.dma_start(out=gamma_b, in_=bcast128(gamma_d))
            bd128 = consts.tile([128, H], fp32)
            nc.gpsimd.dma_start(out=bd128, in_=bcast128(bd_d))
            expm_sb = consts.tile([128, NKC], fp32)
            nc.gpsimd.dma_start(out=expm_sb, in_=expm_d.rearrange("(c p) -> p c", p=128))
            eps_sb = consts.tile([128, 1], fp32)
            nc.vector.memset(eps_sb, EPS)
            # broadcast selector: row 0 -> out partitions 0:64, row 64 -> 64:128.
            # 65-deep so the broadcast matmul stays in fast full-tile mode.
            sel65 = consts.tile([65, 128], bf16)
            nc.vector.memset(sel65, 0.0)
            nc.vector.memset(sel65[0:1, 0:64], 1.0)
            nc.vector.memset(sel65[64:65, 64:128], 1.0)
            # two recip carriers, alternated per attention block so the
            # deferred broadcast matmul of block i never WAR-blocks block i+1
            recip65s = []
            for i in range(2):
                r = consts.tile([65, 512], bf16, name=f"recip65_{i}")
                nc.vector.memset(r, 0.0)
                recip65s.append(r)

            # ctxT[hl*64+d, hp, q] = ctx[q, (hp*2+hl)*64+d] / sumexp
            ctxt = ctxt_pool.tile([128, NPAIR, SQ], bf16)
            wd_sb = ctxt_pool.tile([128, NCI, H], bf16, name="wd_sb")

            ctx_mid = tc.tile_pool(name="midA", bufs=1)
            midA = ctx_mid.__enter__()
            xt = midA.tile([128, NCI, S], bf16, name="xt")
            wk_full = midA.tile([128, NCI, H], bf16, name="wk_full")
            wq_full = midA.tile([128, NCI, H], bf16, name="wq_full")
            ktp_full = midA.tile([128, NCI, S], bf16, name="ktp_full")
            qtp_all = midA.tile([128, NPAIR, SQ], bf16, name="qtp_all")
            v_all = midA.tile([128, NKC, NH, 65], bf16, name="v_all")
            # denominator column: e^{mask_k} per key token, all heads
            nc.gpsimd.tensor_copy(
                out=v_all[:, :, :, 64:65].rearrange("p a b c -> p a (b c)"),
                in_=bass.AP(
                    tensor=expm_sb.tensor,
                    offset=expm_sb.offset,
                    ap=list(expm_sb.ap) + [[0, NH]],
                ),
            )

            # --- phase 1: V for all tokens; K(ci=0) and Q(hp=0) upfront ---
            with (
                tc.tile_pool(name="wvf", bufs=1) as wvf_pool,
                tc.tile_pool(name="vp", bufs=2, space="PSUM") as vp,
            ):
                wv_full = wvf_pool.tile([128, NCI, H], bf16, name="wv_full")
                wv_re = wv_d.rearrange("(c p) n -> p c n", p=128)
                nc.sync.dma_start(out=wv_full[:, :, 0:512], in_=wv_re[:, :, 0:512])
                for tch in range(NKC):
                    eng = (nc.sync, nc.scalar)[tch % 2]
                    eng.dma_start(
                        out=xt[:, :, tch * 128 : (tch + 1) * 128], in_=xtb_d[tch]
                    )
                nc.scalar.dma_start(out=wv_full[:, :, 512:H], in_=wv_re[:, :, 512:H])
                nc.sync.dma_start(
                    out=wq_full, in_=wq_d.rearrange("(c p) n -> p c n", p=128)
                )
                nc.scalar.dma_start(
                    out=wk_full, in_=wk_d.rearrange("(c p) n -> p c n", p=128)
                )
                nc.sync.dma_start(
                    out=wd_sb, in_=wd_d.rearrange("(c p) n -> p c n", p=128)
                )
                # V for own tokens: v_all[t, tb, h, 0:64] *= e^{m_t}
                for nh in range(2):
                    for tb in range(NKC):
                        pv = vp.tile([128, 512], fp32)
                        for ci in range(NCI):
                            nc.tensor.matmul(
                                pv,
                                xt[:, ci, tb * 128 : (tb + 1) * 128],
                                wv_full[:, ci, nh * 512 : (nh + 1) * 512],
                                start=(ci == 0),
                                stop=(ci == NCI - 1),
                            )
                        nc.vector.tensor_scalar_mul(
                            out=v_all[:, tb, nh * 8 : (nh + 1) * 8, 0:64],
                            in0=pv.rearrange("p (a b) -> p a b", a=8),
                            scalar1=expm_sb[:, tb : tb + 1],
                        )
                # K'^T for ci=0 and Q for hp=0 upfront; the rest interleave
                # into the attention kc-loop as PE filler (keeps the PE fed
                # while the exp engines catch up)
                for tb in range(4):
                    pk = vp.tile([128, 512], fp32)
                    for c2 in range(NCI):
                        nc.tensor.matmul(
                            pk,
                            wk_full[:, c2, 0:128],
                            xt[:, c2, tb * 512 : (tb + 1) * 512],
                            start=(c2 == 0),
                            stop=(c2 == NCI - 1),
                        )
                    nc.scalar.activation(
                        out=ktp_full[:, 0, tb * 512 : (tb + 1) * 512],
                        in_=pk,
                        func=AF.Identity,
                        bias=bkT[:, 0:1],
                    )
                for qb in range(SQ // 512):
                    pq = vp.tile([128, 512], fp32)
                    for ci in range(NCI):
                        nc.tensor.matmul(
                            pq,
                            wq_full[:, ci, 0:128],
                            xt[:, ci, qb * 512 : (qb + 1) * 512],
                            start=(ci == 0),
                            stop=(ci == NCI - 1),
                        )
                    nc.scalar.activation(
                        out=qtp_all[:, 0, qb * 512 : (qb + 1) * 512],
                        in_=pq,
                        func=AF.Identity,
                        bias=bqT[:, 0:1],
                    )

            # --- phase 2: attention per head pair ---
            with (
                tc.tile_pool(name="exps", bufs=3) as exps_pool,
                tc.tile_pool(name="exps16", bufs=3) as exps16_pool,
                tc.tile_pool(name="recips", bufs=1) as recips_pool,
                tc.tile_pool(name="pp", bufs=2, space="PSUM") as pp,
                tc.tile_pool(name="sp", bufs=2, space="PSUM") as sp,
                tc.tile_pool(name="cp", bufs=1, space="PSUM") as cp,
            ):
                pending_norm = []

                def k_job(ci, tb):
                    def run():
                        pk = pp.tile([128, 512], fp32, tag="proj")
                        for c2 in range(NCI):
                            nc.tensor.matmul(
                                pk,
                                wk_full[:, c2, ci * 128 : (ci + 1) * 128],
                                xt[:, c2, tb * 512 : (tb + 1) * 512],
                                start=(c2 == 0),
                                stop=(c2 == NCI - 1),
                            )
                        nc.scalar.activation(
                            out=ktp_full[:, ci, tb * 512 : (tb + 1) * 512],
                            in_=pk,
                            func=AF.Identity,
                            bias=bkT[:, ci : ci + 1],
                        )
                    return run

                def q_job(hp2, qb2):
                    def run():
                        pq = pp.tile([128, 512], fp32, tag="proj")
                        for ci in range(NCI):
                            nc.tensor.matmul(
                                pq,
                                wq_full[:, ci, hp2 * 128 : (hp2 + 1) * 128],
                                xt[:, ci, qb2 * 512 : (qb2 + 1) * 512],
                                start=(ci == 0),
                                stop=(ci == NCI - 1),
                            )
                        nc.scalar.activation(
                            out=qtp_all[:, hp2, qb2 * 512 : (qb2 + 1) * 512],
                            in_=pq,
                            func=AF.Identity,
                            bias=bqT[:, hp2 : hp2 + 1],
                        )
                    return run

                proj_jobs = []
                for hp2 in range(1, NPAIR):
                    for tb in range(4):
                        proj_jobs.append(k_job(hp2, tb))
                    for qb2 in range(SQ // 512):
                        proj_jobs.append(q_job(hp2, qb2))
                proj_jobs.reverse()  # pop() takes from the logical front

                for hp in range(NPAIR):
                    # the two heads run concurrently in the PE array (row
                    # groups 0-1 vs 2-3 via tile_position auto-derivation);
                    # ctx lags scores by 3 chunks; normalize part 2 of each
                    # block is deferred into the next block's stream.
                    for qb in range(SQ // 512):
                        qsl = slice(qb * 512, (qb + 1) * 512)
                        pcA = cp.tile([65, 512], fp32, tag="pcA")
                        pcB = cp.tile([65, 512], fp32, tag="pcB")
                        es_q = {}

                        def ctx_pair(kc):
                            for hl in range(2):
                                nc.tensor.matmul(
                                    pcA if hl == 0 else pcB,
                                    v_all[:, kc, hp * 2 + hl, :],
                                    es_q[kc][:, hl, :],
                                    start=(kc == 0),
                                    stop=(kc == NKC - 1),
                                )
                            del es_q[kc]

                        for kc in range(NKC):
                            kcsl = slice(kc * 128, (kc + 1) * 128)
                            ps = sp.tile([128, 2, 512], fp32)
                            nc.tensor.matmul(
                                ps[:, 0, :],
                                ktp_full[0:64, hp, kcsl],
                                qtp_all[0:64, hp, qsl],
                                start=True,
                                stop=True,
                            )
                            nc.tensor.matmul(
                                ps[:, 1, :],
                                ktp_full[64:128, hp, kcsl],
                                qtp_all[64:128, hp, qsl],
                                start=True,
                                stop=True,
                            )
                            if kc == 2 and pending_norm:
                                pending_norm.pop()()
                            if kc in (5, 10, 15) and proj_jobs:
                                proj_jobs.pop()()
                            if kc in DVE_KC:
                                es16 = exps16_pool.tile([128, 2, 512], int16)
                                nc.vector.tensor_scalar(
                                    out=es16.rearrange("p a b -> p (a b)"),
                                    in0=ps.rearrange("p a b -> p (a b)"),
                                    scalar1=SCHR_A,
                                    scalar2=SCHR_B,
                                    op0=OP.mult,
                                    op1=OP.add,
                                )
                                es = es16.bitcast(bf16)
                            else:
                                es = exps_pool.tile([128, 2, 512], bf16)
                                nc.scalar.activation(
                                    out=es.rearrange("p a b -> p (a b)"),
                                    in_=ps.rearrange("p a b -> p (a b)"),
                                    func=AF.Exp,
                                    scale=0.125,
                                )
                            es_q[kc] = es
                            if kc >= 3:
                                ctx_pair(kc - 3)
                        ctx_pair(NKC - 3)
                        ctx_pair(NKC - 2)
                        ctx_pair(NKC - 1)
                        # normalize, part 1 (immediate -- drains pcA/pcB):
                        # denominator rows to SBUF, reciprocal, fill the recip
                        # carrier; ctx rows copied to ctxt on the scalar engine.
                        # (custom-DVE reciprocal needs SBUF base-partition-0
                        # operands on HW -- copy the PSUM denominator row first.)
                        r65 = recip65s[(hp * 2 + qb) % 2]
                        sumsA = recips_pool.tile([1, 512], fp32, tag="sumsA")
                        sumsB = recips_pool.tile([1, 512], fp32, tag="sumsB")
                        nc.vector.tensor_copy(out=sumsA, in_=pcA[64:65, :])
                        nc.vector.tensor_copy(out=sumsB, in_=pcB[64:65, :])
                        recipA = recips_pool.tile([1, 512], fp32, tag="recipA")
                        recipB = recips_pool.tile([1, 512], fp32, tag="recipB")
                        nc.vector.reciprocal_approx_fast(out=recipA, in_=sumsA)
                        nc.vector.reciprocal_approx_fast(out=recipB, in_=sumsB)
                        nc.vector.tensor_copy(out=r65[0:1, :], in_=recipA)
                        nc.vector.tensor_copy(out=r65[64:65, :], in_=recipB)
                        nc.scalar.copy(out=ctxt[0:64, hp, qsl], in_=pcA[0:64, :])
                        nc.scalar.copy(out=ctxt[64:128, hp, qsl], in_=pcB[0:64, :])

                        # normalize, part 2 (deferred): broadcast 1/sum to 128
                        # partitions via one 65-deep matmul, then scale ctxt
                        def norm2(hp=hp, qsl=qsl, r65=r65):
                            pb = pp.tile([128, 512], fp32, tag="proj")
                            nc.tensor.matmul(pb, sel65, r65, start=True, stop=True)
                            nc.vector.tensor_mul(
                                ctxt[:, hp, qsl], ctxt[:, hp, qsl], pb
                            )

                        pending_norm.append(norm2)
                if pending_norm:
                    pending_norm.pop()()

            ctx_mid.__exit__(None, None, None)

            # --- phase 3: output projection + LayerNorm + residual ---
            with (
                tc.tile_pool(name="hid", bufs=3) as hid_pool,
                tc.tile_pool(name="lnbuf", bufs=3) as lnbuf,
                tc.tile_pool(name="op", bufs=3, space="PSUM") as op_pool,
            ):
                for qt in range(SQ // 128):
                    qsl = slice(qt * 128, (qt + 1) * 128)
                    hid = hid_pool.tile([128, H], fp32)
                    for nb in range(2):
                        po = op_pool.tile([128, 512], fp32)
                        for ci in range(NCI):
                            nc.tensor.matmul(
                                po,
                                ctxt[:, ci, qsl],
                                wd_sb[:, ci, nb * 512 : (nb + 1) * 512],
                                start=(ci == 0),
                                stop=(ci == NCI - 1),
                            )
                        nc.vector.tensor_tensor(
                            out=hid[:, nb * 512 : (nb + 1) * 512],
                            in0=po,
                            in1=bd128[:, nb * 512 : (nb + 1) * 512],
                            op=OP.add,
                        )
                    # LayerNorm stats
                    stats = lnbuf.tile([128, 2, 6], fp32, tag="stats")
                    for sg in range(2):
                        nc.vector.bn_stats(
                            out=stats[:, sg, :], in_=hid[:, sg * 512 : (sg + 1) * 512]
                        )
                    mv = lnbuf.tile([128, 2], fp32, tag="mv")
                    nc.vector.bn_aggr(out=mv, in_=stats)
                    rstd = lnbuf.tile([128, 1], fp32, tag="rstd")
                    nc.scalar.activation(
                        out=rstd, in_=mv[:, 1:2], func=AF.Sqrt, bias=eps_sb
                    )
                    nc.vector.reciprocal(rstd, rstd)
                    # residual (beta pre-folded into xkv on the host:
                    # norm + (x + beta) == norm + beta + x exactly)
                    x_res = lnbuf.tile([128, H], fp32, tag="xres")
                    nc.sync.dma_start(out=x_res, in_=xkv[qsl, :])
                    # (hid - mu) * rstd * gamma + (x + beta)
                    norm = lnbuf.tile([128, H], fp32, tag="norm")
                    nc.vector.tensor_scalar(
                        out=norm,
                        in0=hid,
                        scalar1=mv[:, 0:1],
                        scalar2=rstd,
                        op0=OP.subtract,
                        op1=OP.mult,
                    )
                    nc.vector.tensor_mul(norm, norm, gamma_b)
                    final = lnbuf.tile([128, H], fp32, tag="final")
                    nc.gpsimd.tensor_tensor(out=final, in0=norm, in1=x_res, op=OP.add)
                    nc.sync.dma_start(out=out_d[qsl, :], in_=final)

    nc.compile()
    return nc


def get_nc():
    if "nc" not in _cache:
        _cache["nc"] = _build()
    return _cache["nc"]


def make_in_maps(inputs):
    q = np.ascontiguousarray(np.asarray(inputs["query"], dtype=np.float32))
    am = np.asarray(inputs["attention_mask"], dtype=np.float32).reshape(B, S)
    import ml_dtypes

    bfl = ml_dtypes.bfloat16
    # bv folds exactly into the output bias: sum of probs is 1, so
    # (ctx + bv) @ Wd + bd == ctx @ Wd + (bv @ Wd + bd). Computed in fp64.
    bd_eff = (
        np.asarray(inputs["bd"], np.float64)
        + np.asarray(inputs["bv"], np.float64) @ np.asarray(inputs["Wd"], np.float64)
    ).astype(np.float32)
    shared = {
        "wq": np.ascontiguousarray(np.asarray(inputs["Wq"], np.float32).astype(bfl)),
        "wk": np.ascontiguousarray(np.asarray(inputs["Wk"], np.float32).astype(bfl)),
        "wv": np.ascontiguousarray(np.asarray(inputs["Wv"], np.float32).astype(bfl)),
        "wd": np.ascontiguousarray(np.asarray(inputs["Wd"], np.float32).astype(bfl)),
        "bq": np.asarray(inputs["bq"], np.float32),
        "bk": np.asarray(inputs["bk"], np.float32),
        "bd": bd_eff,
        "gamma": np.asarray(inputs["ln_gamma"], np.float32),
    }
    beta_h = np.asarray(inputs["ln_beta"], np.float32)[None, :]
    in_maps = []
    for c in range(NCORES):
        b, hf = c // 2, c % 2
        # queries first, then the other half -- key order is softmax-invariant
        if hf == 0:
            xp = q[b]
            mask = am[b]
        else:
            xp = np.concatenate([q[b, SQ:], q[b, :SQ]], axis=0)
            mask = np.concatenate([am[b, SQ:], am[b, :SQ]], axis=0)
        m = dict(shared)
        m["xkv"] = np.ascontiguousarray(xp[0:SQ] + beta_h)
        xtc = xp.reshape(S // 128, 128, H // 128, 128).transpose(0, 3, 2, 1)
        m["xtb"] = np.ascontiguousarray(xtc.astype(bfl))
        m["expm"] = np.ascontiguousarray(np.exp(mask.astype(np.float64)).astype(np.float32))
        in_maps.append(m)
    return in_maps


def assemble(results):
    out = np.empty((B, S, H), dtype=np.float32)
    for c in range(NCORES):
        b, hf = c // 2, c % 2
        out[b, hf * SQ : (hf + 1) * SQ, :] = results[c]["out"]
    return out


def kernel(**inputs):
    from concourse.bass_utils import run_bass_kernel_spmd

    nc = get_nc()
    in_maps = make_in_maps(inputs)
    res = run_bass_kernel_spmd(nc, in_maps, core_ids=list(range(NCORES)))
    return assemble(res.results)


if __name__ == "__main__":
    rng = np.random.default_rng(0)
    inputs = {
        "query": rng.standard_normal((B, S, H), dtype=np.float32),
        "attention_mask": np.zeros((B, 1, 1, S), np.float32),
        "Wq": rng.standard_normal((H, H), dtype=np.float32) * 0.02,
        "bq": np.zeros(H, np.float32),
        "Wk": rng.standard_normal((H, H), dtype=np.float32) * 0.02,
        "bk": np.zeros(H, np.float32),
        "Wv": rng.standard_normal((H, H), dtype=np.float32) * 0.02,
        "bv": np.zeros(H, np.float32),
        "Wd": rng.standard_normal((H, H), dtype=np.float32) * 0.02,
        "bd": np.zeros(H, np.float32),
        "ln_gamma": np.ones(H, np.float32),
        "ln_beta": np.zeros(H, np.float32),
    }
    out = kernel(**inputs)
    print(out.shape, out.dtype)
